# revision 1
# baseline (speedup 1.0000x reference)
"""Trainium2 Bass kernel for a custom LSTM cell.

reference:
    z = concat([h_tm1, inputs], -1) @ kernel      # [B, 4U]
    i, f, g, o = split(z, 4, -1)
    c = sigmoid(f) * c_tm1 + sigmoid(i) * tanh(g)
    h = sigmoid(o) * tanh(c)
    returns (h, c)

Sharding over 8 NeuronCores: 2-way over batch x 4-way over units
(each gate's block co-located per core).  Per core:
    z_blk = A_half @ W[:, 4 gate slices of 256] via fp32r matmuls
    (TF32-like, full PE rate), gate math on-chip, outputs [1024, 256]
    h/c blocks.  Host only slices/concatenates.

Schedule (per core):
  phase 1: k-outer round-robin over 8 open PSUM groups = all 8 batch
           sub-tiles x (i|f) columns, consuming at/wk chunks in DMA
           arrival order so the PE stays busy and HAM-warm during the
           load window.  Each group closes with one Sigmoid -> sig_if,
           freeing its PSUM bank.
  phase 2: per-m serial accumulation of (g|o) columns from SBUF-resident
           data, epilogue (tanh/sigmoid/DVE combine) pipelined per m.
Inputs stream on the Sync queue in consumption order; outputs go out on
the GpSimd queue.
"""

import sys

sys.path.insert(0, "/opt/trn_rl_repo")

import numpy as np

BATCH, INPUT_DIM, UNITS = 2048, 512, 1024
K = UNITS + INPUT_DIM  # contraction dim, 1536
R, C = 2, 4  # batch halves x unit quarters
BR = BATCH // R  # 1024 batch rows per core
UC = UNITS // C  # 256 units per core
KS = K // 128  # 12 k-subtiles
M = BR // 128  # 8 batch sub-chunks per core

_CACHE = {}


def _build_nc():
    import concourse.tile as tile
    from concourse import bacc, mybir

    f32 = mybir.dt.float32
    f32r = mybir.dt.float32r
    Sig = mybir.ActivationFunctionType.Sigmoid
    Tanh = mybir.ActivationFunctionType.Tanh

    nc = bacc.Bacc("TRN2")
    at_in = nc.declare_dram_parameter("at", [K, BR], f32r, isOutput=False)
    wklo_in = nc.declare_dram_parameter("wk_lo", [K, 512], f32r, isOutput=False)
    wkhi_in = nc.declare_dram_parameter("wk_hi", [K, 512], f32r, isOutput=False)
    ct_in = nc.declare_dram_parameter("ct", [BR, UC], f32, isOutput=False)
    h_out = nc.declare_dram_parameter("h_out", [BR, UC], f32, isOutput=True)
    c_out = nc.declare_dram_parameter("c_out", [BR, UC], f32, isOutput=True)

    with tile.TileContext(nc) as tc:
        with (
            tc.tile_pool(name="data", bufs=1) as data,
            tc.tile_pool(name="work", bufs=3) as work,
            tc.tile_pool(name="psum", bufs=8, space="PSUM") as psum,
        ):
            at = data.tile([128, KS, BR], f32r)
            wk_lo = data.tile([128, KS, 512], f32r)  # i|f columns
            wk_hi = data.tile([128, KS, 512], f32r)  # g|o columns
            ct = data.tile([128, M, UC], f32)
            sig_if = data.tile([128, M, 512], f32)
            fc_all = data.tile([128, M, UC], f32)

            at_r = at_in[:].rearrange("(ko p) n -> p ko n", p=128)
            wklo_r = wklo_in[:].rearrange("(ko p) n -> p ko n", p=128)
            wkhi_r = wkhi_in[:].rearrange("(ko p) n -> p ko n", p=128)
            ct_r = ct_in[:].rearrange("(m p) u -> p m u", p=128)

            # inputs on the Sync queue in consumption order.  First chunks are
            # single-k so the PE can start early; the cold (HAM-throttled) PE
            # naturally paces with the DMA ramp.  The very first at chunk is
            # halved by columns so k=0/m0-3 matmuls can fire earliest.
            nc.sync.dma_start(at[:, 0:1, 0:512], at_r[:, 0:1, 0:512])
            nc.sync.dma_start(wk_lo[:, 0:1, :], wklo_r[:, 0:1, :])
            nc.sync.dma_start(at[:, 0:1, 512:1024], at_r[:, 0:1, 512:1024])
            chunks = [slice(j, j + 1) for j in range(1, KS)]
            for ks in chunks:
                nc.sync.dma_start(at[:, ks, :], at_r[:, ks, :])
                nc.sync.dma_start(wk_lo[:, ks, :], wklo_r[:, ks, :])
            chunks = [slice(j, j + 1) for j in range(KS)]
            for ks in chunks:
                nc.sync.dma_start(wk_hi[:, ks, :], wkhi_r[:, ks, :])
            for j in range(M // 2):
                ms2 = slice(2 * j, 2 * j + 2)
                nc.sync.dma_start(ct[:, ms2, :], ct_r[:, ms2, :])

            # phase 1: all m, i|f columns, k-outer round-robin
            plo = [
                psum.tile([128, 512], f32, tag="ps", name=f"plo{m}") for m in range(M)
            ]
            for k in range(KS):
                for m in range(M):
                    nc.tensor.matmul(
                        plo[m][:],
                        at[:, k, m * 128 : (m + 1) * 128],
                        wk_lo[:, k, :],
                        start=(k == 0),
                        stop=(k == KS - 1),
                    )
            for m in range(M):
                nc.scalar.activation(sig_if[:, m, :], plo[m][:], Sig)
            # f * c_tm1 off the epilogue critical path (DVE is idle here)
            for m in range(M):
                nc.vector.tensor_mul(
                    fc_all[:, m, :], sig_if[:, m, UC : 2 * UC], ct[:, m, :]
                )

            # phase 2: per-m serial g|o accumulation + epilogue
            for m in range(M):
                ms = slice(m * 128, (m + 1) * 128)
                phi = psum.tile([128, 512], f32, tag="ps", name=f"phi{m}")
                for k in range(KS):
                    nc.tensor.matmul(
                        phi[:],
                        at[:, k, ms],
                        wk_hi[:, k, :],
                        start=(k == 0),
                        stop=(k == KS - 1),
                    )
                tg = work.tile([128, UC], f32, tag="tg")
                nc.scalar.activation(tg[:], phi[:, 0:UC], Tanh)
                so = work.tile([128, UC], f32, tag="so")
                nc.scalar.activation(so[:], phi[:, UC : 2 * UC], Sig)
                ig = work.tile([128, UC], f32, tag="ig")
                nc.vector.tensor_mul(ig[:], sig_if[:, m, 0:UC], tg[:])
                cn = work.tile([128, UC], f32, tag="cn")
                nc.vector.tensor_add(cn[:], fc_all[:, m, :], ig[:])
                th = work.tile([128, UC], f32, tag="th")
                nc.scalar.activation(th[:], cn[:], Tanh)
                hn = work.tile([128, UC], f32, tag="hn")
                nc.vector.tensor_mul(hn[:], so[:], th[:])
                nc.gpsimd.dma_start(c_out[ms, :], cn[:])
                nc.gpsimd.dma_start(h_out[ms, :], hn[:])

    nc.compile()
    return nc


def get_nc():
    if "nc" not in _CACHE:
        _CACHE["nc"] = _build_nc()
    return _CACHE["nc"]


def make_in_maps(inputs, h_tm1, c_tm1, kernel):
    x = np.ascontiguousarray(np.asarray(inputs, dtype=np.float32))
    h = np.ascontiguousarray(np.asarray(h_tm1, dtype=np.float32))
    c = np.ascontiguousarray(np.asarray(c_tm1, dtype=np.float32))
    w = np.ascontiguousarray(np.asarray(kernel, dtype=np.float32))
    at_full = np.ascontiguousarray(np.concatenate([h, x], axis=1).T)  # [K, B]
    in_maps = []
    for core in range(R * C):
        r, ci = divmod(core, C)
        at_np = np.ascontiguousarray(at_full[:, r * BR : (r + 1) * BR])
        gates = [
            w[:, g * UNITS + ci * UC : g * UNITS + (ci + 1) * UC] for g in range(4)
        ]
        wklo_np = np.ascontiguousarray(np.concatenate(gates[0:2], axis=1))
        wkhi_np = np.ascontiguousarray(np.concatenate(gates[2:4], axis=1))
        ct_np = np.ascontiguousarray(c[r * BR : (r + 1) * BR, ci * UC : (ci + 1) * UC])
        in_maps.append(
            {"at": at_np, "wk_lo": wklo_np, "wk_hi": wkhi_np, "ct": ct_np}
        )
    return in_maps


def assemble(results):
    h_new = np.empty((BATCH, UNITS), dtype=np.float32)
    c_new = np.empty((BATCH, UNITS), dtype=np.float32)
    for core in range(R * C):
        r, ci = divmod(core, C)
        h_new[r * BR : (r + 1) * BR, ci * UC : (ci + 1) * UC] = results[core]["h_out"]
        c_new[r * BR : (r + 1) * BR, ci * UC : (ci + 1) * UC] = results[core]["c_out"]
    return h_new, c_new


def kernel(inputs, h_tm1, c_tm1, kernel):
    from concourse.bass_utils import run_bass_kernel_spmd

    nc = get_nc()
    in_maps = make_in_maps(inputs, h_tm1, c_tm1, kernel)
    res = run_bass_kernel_spmd(nc, in_maps, list(range(R * C)), trace=False)
    return assemble(res.results)



# revision 3
# speedup vs baseline: 1.1641x; 1.1641x over previous
"""Trainium2 Bass kernel for a custom LSTM cell.

reference:
    z = concat([h_tm1, inputs], -1) @ kernel      # [B, 4U]
    i, f, g, o = split(z, 4, -1)
    c = sigmoid(f) * c_tm1 + sigmoid(i) * tanh(g)
    h = sigmoid(o) * tanh(c)
    returns (h, c)

Sharding over 8 NeuronCores: 2-way over batch x 4-way over units
(each gate's block co-located per core).  Per core:
    z_blk = A_half @ W[:, 4 gate slices of 256] in fp16 (matmul inputs
    quantized host-side; fp16 keeps h rel-err ~2e-3, well under 2e-2,
    while halving HBM traffic vs fp32r at the same PE rate).

Schedule (per core):
  warmup: dummy matmuls ramp the PE p-state clock while the first DMA
          chunks are in flight; one dummy Sigmoid preloads the Act
          table that serves both Sigmoid and Tanh.
  phase 1: k-outer round-robin over 8 PSUM groups = 8 batch sub-tiles
           x (i|f) columns, consuming at/wk_lo chunks in DMA arrival
           order.  Each group closes with one Sigmoid -> sig_if, then
           f*c_tm1 on the DVE, freeing its PSUM bank.
  phase 2: per-m serial accumulation of g then o columns (two 256-col
           groups in one bank) so tanh(g) overlaps the o matmuls;
           epilogue (tanh/sigmoid/DVE combine) pipelined per m.
Inputs stream on the Sync queue in consumption order (k-pair chunks
so HWDGE descriptor gen stays off the critical path); outputs also go
out on the Sync queue (HWDGE) to keep the Pool engine's slow SWDGE
descriptor generation off the tail.
"""

import sys

sys.path.insert(0, "/opt/trn_rl_repo")

import numpy as np

BATCH, INPUT_DIM, UNITS = 2048, 512, 1024
K = UNITS + INPUT_DIM  # contraction dim, 1536
R, C = 2, 4  # batch halves x unit quarters
BR = BATCH // R  # 1024 batch rows per core
UC = UNITS // C  # 256 units per core
KS = K // 128  # 12 k-subtiles
M = BR // 128  # 8 batch sub-chunks per core

_CACHE = {}


def _build_nc():
    import concourse.tile as tile
    from concourse import bacc, mybir

    f32 = mybir.dt.float32
    f16 = mybir.dt.float16
    Sig = mybir.ActivationFunctionType.Sigmoid
    Tanh = mybir.ActivationFunctionType.Tanh

    nc = bacc.Bacc("TRN2")
    at_in = nc.declare_dram_parameter("at", [K, BR], f16, isOutput=False)
    wklo_in = nc.declare_dram_parameter("wk_lo", [K, 512], f16, isOutput=False)
    wkhi_in = nc.declare_dram_parameter("wk_hi", [K, 512], f16, isOutput=False)
    ct_in = nc.declare_dram_parameter("ct", [BR, UC], f32, isOutput=False)
    h_out = nc.declare_dram_parameter("h_out", [BR, UC], f32, isOutput=True)
    c_out = nc.declare_dram_parameter("c_out", [BR, UC], f32, isOutput=True)

    with tile.TileContext(nc) as tc:
        with (
            tc.tile_pool(name="data", bufs=1) as data,
            tc.tile_pool(name="work", bufs=3) as work,
            tc.tile_pool(name="psum", bufs=8, space="PSUM") as psum,
        ):
            at = data.tile([128, KS, BR], f16)
            wk_lo = data.tile([128, KS, 512], f16)  # i|f columns
            wk_hi = data.tile([128, KS, 512], f16)  # g|o columns
            ct = data.tile([128, M, UC], f32)
            sig_if = data.tile([128, M, 512], f32)
            fc_all = data.tile([128, M, UC], f32)
            # warmup fodder: never written, contents irrelevant (results
            # are discarded / overwritten by start=True groups)
            dum_w = data.tile([128, 128], f16)
            dum_m = data.tile([128, 512], f16)
            dum_a = data.tile([128, 8], f32)
            dum_o = data.tile([128, 8], f32)

            at_r = at_in[:].rearrange("(ko p) n -> p ko n", p=128)
            wklo_r = wklo_in[:].rearrange("(ko p) n -> p ko n", p=128)
            wkhi_r = wkhi_in[:].rearrange("(ko p) n -> p ko n", p=128)
            ct_r = ct_in[:].rearrange("(m p) u -> p m u", p=128)

            plo = [
                psum.tile([128, 512], f32, tag="ps", name=f"plo{m}") for m in range(M)
            ]

            # PE p-state warmup: keep the PE continuously busy from t~0 so
            # the 2.4GHz p-state is reached by the time real data lands.
            nc.gpsimd.memset(dum_w[:], 0.0)
            nc.gpsimd.memset(dum_m[:], 0.0)
            nc.gpsimd.memset(dum_a[:], 0.0)
            for _ in range(10):
                nc.tensor.matmul(
                    plo[0][:], dum_w[:], dum_m[:], start=True, stop=True,
                    skip_group_check=True,
                )
            # Act table preload (Sigmoid's table also serves Tanh)
            nc.scalar.activation(dum_o[:], dum_a[:], Sig)

            # inputs on the Sync queue in consumption order.  k0/k1 go as
            # single-k chunks so the PE can start early; later k's ship as
            # k-pairs so HWDGE descriptor generation (625ns/DMA) stays
            # ahead of the transfers.
            nc.sync.dma_start(at[:, 0:1, 0:512], at_r[:, 0:1, 0:512])
            nc.sync.dma_start(wk_lo[:, 0:1, :], wklo_r[:, 0:1, :])
            nc.sync.dma_start(at[:, 0:1, 512:1024], at_r[:, 0:1, 512:1024])
            nc.sync.dma_start(wk_hi[:, 0:1, :], wkhi_r[:, 0:1, :])
            nc.sync.dma_start(at[:, 1:2, :], at_r[:, 1:2, :])
            nc.sync.dma_start(wk_lo[:, 1:2, :], wklo_r[:, 1:2, :])
            nc.sync.dma_start(wk_hi[:, 1:2, :], wkhi_r[:, 1:2, :])
            for j in range(1, KS // 2):
                ks = slice(2 * j, 2 * j + 2)
                nc.sync.dma_start(at[:, ks, :], at_r[:, ks, :])
                nc.sync.dma_start(wk_lo[:, ks, :], wklo_r[:, ks, :])
                nc.sync.dma_start(wk_hi[:, ks, :], wkhi_r[:, ks, :])
            for j in range(2):
                ms4 = slice(4 * j, 4 * j + 4)
                nc.sync.dma_start(ct[:, ms4, :], ct_r[:, ms4, :])

            # phase 1: all m, i|f columns, k-outer round-robin
            for k in range(KS):
                for m in range(M):
                    nc.tensor.matmul(
                        plo[m][:],
                        at[:, k, m * 128 : (m + 1) * 128],
                        wk_lo[:, k, :],
                        start=(k == 0),
                        stop=(k == KS - 1),
                    )
            for m in range(M):
                nc.scalar.activation(sig_if[:, m, :], plo[m][:], Sig)
            # f * c_tm1 off the epilogue critical path (DVE is idle here)
            for m in range(M):
                nc.vector.tensor_mul(
                    fc_all[:, m, :], sig_if[:, m, UC : 2 * UC], ct[:, m, :]
                )

            # phase 2: per-m serial g|o accumulation + epilogue.  g and o
            # are separate 256-col groups in one bank so tanh(g) and the
            # downstream DVE chain overlap the o-column matmuls.
            for m in range(M):
                ms = slice(m * 128, (m + 1) * 128)
                phi = psum.tile([128, 512], f32, tag="ps", name=f"phi{m}")
                for k in range(KS):
                    nc.tensor.matmul(
                        phi[:, 0:UC],
                        at[:, k, ms],
                        wk_hi[:, k, 0:UC],
                        start=(k == 0),
                        stop=(k == KS - 1),
                    )
                tg = work.tile([128, UC], f32, tag="tg")
                nc.scalar.activation(tg[:], phi[:, 0:UC], Tanh)
                for k in range(KS):
                    nc.tensor.matmul(
                        phi[:, UC : 2 * UC],
                        at[:, k, ms],
                        wk_hi[:, k, UC : 2 * UC],
                        start=(k == 0),
                        stop=(k == KS - 1),
                    )
                so = work.tile([128, UC], f32, tag="so")
                nc.scalar.activation(so[:], phi[:, UC : 2 * UC], Sig)
                ig = work.tile([128, UC], f32, tag="ig")
                nc.vector.tensor_mul(ig[:], sig_if[:, m, 0:UC], tg[:])
                cn = work.tile([128, UC], f32, tag="cn")
                nc.vector.tensor_add(cn[:], fc_all[:, m, :], ig[:])
                th = work.tile([128, UC], f32, tag="th")
                nc.scalar.activation(th[:], cn[:], Tanh)
                hn = work.tile([128, UC], f32, tag="hn")
                nc.vector.tensor_mul(hn[:], so[:], th[:])
                nc.sync.dma_start(c_out[ms, :], cn[:])
                nc.sync.dma_start(h_out[ms, :], hn[:])

    nc.compile()
    return nc


def get_nc():
    if "nc" not in _CACHE:
        _CACHE["nc"] = _build_nc()
    return _CACHE["nc"]


def make_in_maps(inputs, h_tm1, c_tm1, kernel):
    x = np.asarray(inputs, dtype=np.float32)
    h = np.asarray(h_tm1, dtype=np.float32)
    c = np.ascontiguousarray(np.asarray(c_tm1, dtype=np.float32))
    w16 = np.asarray(kernel, dtype=np.float32).astype(np.float16)
    at_full = np.ascontiguousarray(
        np.concatenate([h, x], axis=1).T.astype(np.float16)
    )  # [K, B] fp16
    in_maps = []
    for core in range(R * C):
        r, ci = divmod(core, C)
        at_np = np.ascontiguousarray(at_full[:, r * BR : (r + 1) * BR])
        gates = [
            w16[:, g * UNITS + ci * UC : g * UNITS + (ci + 1) * UC] for g in range(4)
        ]
        wklo_np = np.ascontiguousarray(np.concatenate(gates[0:2], axis=1))
        wkhi_np = np.ascontiguousarray(np.concatenate(gates[2:4], axis=1))
        ct_np = np.ascontiguousarray(c[r * BR : (r + 1) * BR, ci * UC : (ci + 1) * UC])
        in_maps.append(
            {"at": at_np, "wk_lo": wklo_np, "wk_hi": wkhi_np, "ct": ct_np}
        )
    return in_maps


def assemble(results):
    h_new = np.empty((BATCH, UNITS), dtype=np.float32)
    c_new = np.empty((BATCH, UNITS), dtype=np.float32)
    for core in range(R * C):
        r, ci = divmod(core, C)
        h_new[r * BR : (r + 1) * BR, ci * UC : (ci + 1) * UC] = results[core]["h_out"]
        c_new[r * BR : (r + 1) * BR, ci * UC : (ci + 1) * UC] = results[core]["c_out"]
    return h_new, c_new


def kernel(inputs, h_tm1, c_tm1, kernel):
    from concourse.bass_utils import run_bass_kernel_spmd

    nc = get_nc()
    in_maps = make_in_maps(inputs, h_tm1, c_tm1, kernel)
    res = run_bass_kernel_spmd(nc, in_maps, list(range(R * C)), trace=False)
    return assemble(res.results)


# revision 5
# speedup vs baseline: 1.1706x; 1.0056x over previous
"""Trainium2 Bass kernel for a custom LSTM cell.

reference:
    z = concat([h_tm1, inputs], -1) @ kernel      # [B, 4U]
    i, f, g, o = split(z, 4, -1)
    c = sigmoid(f) * c_tm1 + sigmoid(i) * tanh(g)
    h = sigmoid(o) * tanh(c)
    returns (h, c)

Sharding over 8 NeuronCores: 2-way over batch x 4-way over units
(each gate's block co-located per core).  Per core:
    z_blk = A_half @ W[:, 4 gate slices of 256] in fp16 (matmul inputs
    quantized host-side; fp16 keeps h rel-err ~2e-3, well under 2e-2,
    while halving HBM traffic vs fp32r at the same PE rate).

Schedule (per core):
  warmup: small dummy matmuls keep the PE continuously busy from ~0.5us
          so the p-state clock is at 2.4GHz when real data lands (~3.6us);
          one dummy Sigmoid preloads the Act table (serves Tanh too).
  phase 1: k-outer round-robin over 8 PSUM groups = 8 batch sub-tiles
           x (i|f) columns, consuming at/wk_lo chunks in DMA arrival
           order.  Each group closes with one Sigmoid -> sig_if, then
           f*c_tm1 on the DVE, freeing its PSUM bank.
  phase 2: per-m serial accumulation of g then o columns in separate
           PSUM tiles (avoids a false WAR dep between tanh(g) and the
           o-group); epilogue pipelined per m.  The last m's o columns
           are split 128/128 so the final act->mul->DMA chain is half
           width (shorter tail).
Inputs stream on the Sync queue in consumption order (k-pair chunks
keep HWDGE descriptor gen ahead of the transfers); outputs also leave
on the Sync queue (HWDGE) to keep Pool's slow SWDGE off the tail.
"""

import sys

sys.path.insert(0, "/opt/trn_rl_repo")

import numpy as np

BATCH, INPUT_DIM, UNITS = 2048, 512, 1024
K = UNITS + INPUT_DIM  # contraction dim, 1536
R, C = 2, 4  # batch halves x unit quarters
BR = BATCH // R  # 1024 batch rows per core
UC = UNITS // C  # 256 units per core
KS = K // 128  # 12 k-subtiles
M = BR // 128  # 8 batch sub-chunks per core

_CACHE = {}


def _build_nc():
    import concourse.tile as tile
    from concourse import bacc, mybir

    f32 = mybir.dt.float32
    f16 = mybir.dt.float16
    Sig = mybir.ActivationFunctionType.Sigmoid
    Tanh = mybir.ActivationFunctionType.Tanh

    nc = bacc.Bacc("TRN2")
    at_in = nc.declare_dram_parameter("at", [K, BR], f16, isOutput=False)
    wklo_in = nc.declare_dram_parameter("wk_lo", [K, 512], f16, isOutput=False)
    wkhi_in = nc.declare_dram_parameter("wk_hi", [K, 512], f16, isOutput=False)
    ct_in = nc.declare_dram_parameter("ct", [BR, UC], f32, isOutput=False)
    h_out = nc.declare_dram_parameter("h_out", [BR, UC], f32, isOutput=True)
    c_out = nc.declare_dram_parameter("c_out", [BR, UC], f32, isOutput=True)

    with tile.TileContext(nc) as tc:
        with (
            tc.tile_pool(name="data", bufs=1) as data,
            tc.tile_pool(name="work", bufs=3) as work,
            tc.tile_pool(name="psum", bufs=8, space="PSUM") as psum,
        ):
            at = data.tile([128, KS, BR], f16)
            wk_lo = data.tile([128, KS, 512], f16)  # i|f columns
            wk_hi = data.tile([128, KS, 512], f16)  # g|o columns
            ct = data.tile([128, M, UC], f32)
            sig_if = data.tile([128, M, 512], f32)
            fc_all = data.tile([128, M, UC], f32)
            # warmup fodder (zeroed; results discarded / overwritten by
            # start=True groups)
            dum_w = data.tile([128, 128], f16)
            dum_a = data.tile([128, 8], f32)
            dum_o = data.tile([128, 8], f32)

            at_r = at_in[:].rearrange("(ko p) n -> p ko n", p=128)
            wklo_r = wklo_in[:].rearrange("(ko p) n -> p ko n", p=128)
            wkhi_r = wkhi_in[:].rearrange("(ko p) n -> p ko n", p=128)
            ct_r = ct_in[:].rearrange("(m p) u -> p m u", p=128)

            plo = [
                psum.tile([128, 512], f32, tag="ps", name=f"plo{m}") for m in range(M)
            ]

            # PE p-state warmup: 128-wide dummy matmuls from ~0.5us until
            # real data lands (~3.6us), so real matmuls start at full clock.
            nc.vector.memset(dum_w[:], 0.0)
            for _ in range(32):
                nc.tensor.matmul(
                    plo[0][:, 0:128], dum_w[:], dum_w[:], start=True, stop=True,
                    skip_group_check=True,
                )
            # Act table preload (Sigmoid's table also serves Tanh)
            nc.gpsimd.memset(dum_a[:], 0.0)
            nc.scalar.activation(dum_o[:], dum_a[:], Sig)

            # inputs on the Sync queue in consumption order.  k0/k1 go as
            # single-k chunks so the PE can start early; later k's ship as
            # k-pairs so HWDGE descriptor generation (625ns/DMA) stays
            # ahead of the transfers.
            nc.sync.dma_start(at[:, 0:1, 0:512], at_r[:, 0:1, 0:512])
            nc.sync.dma_start(wk_lo[:, 0:1, :], wklo_r[:, 0:1, :])
            nc.sync.dma_start(at[:, 0:1, 512:1024], at_r[:, 0:1, 512:1024])
            nc.sync.dma_start(wk_hi[:, 0:1, :], wkhi_r[:, 0:1, :])
            nc.sync.dma_start(at[:, 1:2, :], at_r[:, 1:2, :])
            nc.sync.dma_start(wk_lo[:, 1:2, :], wklo_r[:, 1:2, :])
            nc.sync.dma_start(wk_hi[:, 1:2, :], wkhi_r[:, 1:2, :])
            for j in range(1, KS // 2):
                ks = slice(2 * j, 2 * j + 2)
                nc.sync.dma_start(at[:, ks, :], at_r[:, ks, :])
                nc.sync.dma_start(wk_lo[:, ks, :], wklo_r[:, ks, :])
                nc.sync.dma_start(wk_hi[:, ks, :], wkhi_r[:, ks, :])
            for j in range(2):
                ms4 = slice(4 * j, 4 * j + 4)
                nc.sync.dma_start(ct[:, ms4, :], ct_r[:, ms4, :])

            # phase 1: all m, i|f columns, k-outer round-robin
            for k in range(KS):
                for m in range(M):
                    nc.tensor.matmul(
                        plo[m][:],
                        at[:, k, m * 128 : (m + 1) * 128],
                        wk_lo[:, k, :],
                        start=(k == 0),
                        stop=(k == KS - 1),
                    )
            for m in range(M):
                nc.scalar.activation(sig_if[:, m, :], plo[m][:], Sig)
            # f * c_tm1 off the epilogue critical path (DVE is idle here)
            for m in range(M):
                nc.vector.tensor_mul(
                    fc_all[:, m, :], sig_if[:, m, UC : 2 * UC], ct[:, m, :]
                )

            # phase 2: per-m serial g then o accumulation + epilogue.
            # g and o live in separate PSUM tiles so tanh(g) can't block
            # the o matmuls.  Last m splits o into 128-col halves to
            # shorten the final act->mul->DMA chain.
            for m in range(M):
                last = m == M - 1
                ms = slice(m * 128, (m + 1) * 128)
                phg = psum.tile([128, 512], f32, tag="ps", name=f"phg{m}")
                for k in range(KS):
                    nc.tensor.matmul(
                        phg[:, 0:UC],
                        at[:, k, ms],
                        wk_hi[:, k, 0:UC],
                        start=(k == 0),
                        stop=(k == KS - 1),
                    )
                tg = work.tile([128, UC], f32, tag="tg")
                nc.scalar.activation(tg[:], phg[:, 0:UC], Tanh)

                pho = psum.tile([128, 512], f32, tag="ps", name=f"pho{m}")
                osplits = (
                    [slice(0, 128), slice(128, 256)] if last else [slice(0, UC)]
                )
                for osl in osplits:
                    for k in range(KS):
                        nc.tensor.matmul(
                            pho[:, osl],
                            at[:, k, ms],
                            wk_hi[:, k, UC + osl.start : UC + osl.stop],
                            start=(k == 0),
                            stop=(k == KS - 1),
                        )
                ig = work.tile([128, UC], f32, tag="ig")
                nc.vector.tensor_mul(ig[:], sig_if[:, m, 0:UC], tg[:])
                cn = work.tile([128, UC], f32, tag="cn")
                nc.vector.tensor_add(cn[:], fc_all[:, m, :], ig[:])
                nc.sync.dma_start(c_out[ms, :], cn[:])
                th = work.tile([128, UC], f32, tag="th")
                nc.scalar.activation(th[:], cn[:], Tanh)
                so = work.tile([128, UC], f32, tag="so")
                hn = work.tile([128, UC], f32, tag="hn")
                for osl in osplits:
                    nc.scalar.activation(so[:, osl], pho[:, osl], Sig)
                    nc.vector.tensor_mul(hn[:, osl], so[:, osl], th[:, osl])
                    nc.sync.dma_start(h_out[ms, osl], hn[:, osl])

    nc.compile()
    return nc


def get_nc():
    if "nc" not in _CACHE:
        _CACHE["nc"] = _build_nc()
    return _CACHE["nc"]


def make_in_maps(inputs, h_tm1, c_tm1, kernel):
    x = np.asarray(inputs, dtype=np.float32)
    h = np.asarray(h_tm1, dtype=np.float32)
    c = np.ascontiguousarray(np.asarray(c_tm1, dtype=np.float32))
    w16 = np.asarray(kernel, dtype=np.float32).astype(np.float16)
    at_full = np.ascontiguousarray(
        np.concatenate([h, x], axis=1).T.astype(np.float16)
    )  # [K, B] fp16
    in_maps = []
    for core in range(R * C):
        r, ci = divmod(core, C)
        at_np = np.ascontiguousarray(at_full[:, r * BR : (r + 1) * BR])
        gates = [
            w16[:, g * UNITS + ci * UC : g * UNITS + (ci + 1) * UC] for g in range(4)
        ]
        wklo_np = np.ascontiguousarray(np.concatenate(gates[0:2], axis=1))
        wkhi_np = np.ascontiguousarray(np.concatenate(gates[2:4], axis=1))
        ct_np = np.ascontiguousarray(c[r * BR : (r + 1) * BR, ci * UC : (ci + 1) * UC])
        in_maps.append(
            {"at": at_np, "wk_lo": wklo_np, "wk_hi": wkhi_np, "ct": ct_np}
        )
    return in_maps


def assemble(results):
    h_new = np.empty((BATCH, UNITS), dtype=np.float32)
    c_new = np.empty((BATCH, UNITS), dtype=np.float32)
    for core in range(R * C):
        r, ci = divmod(core, C)
        h_new[r * BR : (r + 1) * BR, ci * UC : (ci + 1) * UC] = results[core]["h_out"]
        c_new[r * BR : (r + 1) * BR, ci * UC : (ci + 1) * UC] = results[core]["c_out"]
    return h_new, c_new


def kernel(inputs, h_tm1, c_tm1, kernel):
    from concourse.bass_utils import run_bass_kernel_spmd

    nc = get_nc()
    in_maps = make_in_maps(inputs, h_tm1, c_tm1, kernel)
    res = run_bass_kernel_spmd(nc, in_maps, list(range(R * C)), trace=False)
    return assemble(res.results)


# revision 7
# speedup vs baseline: 1.2162x; 1.0389x over previous
"""Trainium2 Bass kernel for a custom LSTM cell.

reference:
    z = concat([h_tm1, inputs], -1) @ kernel      # [B, 4U]
    i, f, g, o = split(z, 4, -1)
    c = sigmoid(f) * c_tm1 + sigmoid(i) * tanh(g)
    h = sigmoid(o) * tanh(c)
    returns (h, c)

Sharding over 8 NeuronCores: 2-way over batch x 4-way over units
(each gate's block co-located per core).  Matmul inputs are quantized
to fp16 host-side (h rel-err ~2e-3, well under the 2e-2 budget) which
halves HBM traffic vs fp32r at the same PE rate.

Per core the activations and weights are packed into ONE dram tensor
`awk` [K, 2048] = [at batch 1024 | wk_lo 512 | wk_hi 512] per k-row,
so each k-subtile arrives as a single DMA (descriptor generation at
~650ns/DMA would otherwise outpace the transfers).

Schedule (per core):
  warmup: small dummy matmuls keep the PE busy from ~0.5us so the
          p-state clock is at 2.4GHz when real data lands (~3.9us);
          a dummy Sigmoid preloads the Act table (serves Tanh too).
  phase 1: k-outer round-robin over 8 PSUM groups (batch sub-tiles x
           i|f columns) consuming awk chunks in arrival order; each
           group closes with Sigmoid -> sig_if, then f*c_tm1 on DVE.
  phase 2: m7's g-group runs FIRST so its tanh/ig/cn/tanh(c) chain and
           c_out DMA complete ~17us before the end; then m0..m6
           (g then o per m, separate PSUM tiles to avoid false WAR
           deps), with m7's o-gate last, split into two 128-col groups
           so the final sigmoid->mul->DMA tail is half width.
All input DMAs stream on the Sync queue in consumption order; wk_hi's
k0 block and c_tm1 ride at the end of the stream (first needed by
phase 2).  Outputs also leave on the Sync queue (HWDGE descriptor
generation, keeping Pool's slow SWDGE off the tail).
"""

import sys

sys.path.insert(0, "/opt/trn_rl_repo")

import numpy as np

BATCH, INPUT_DIM, UNITS = 2048, 512, 1024
K = UNITS + INPUT_DIM  # contraction dim, 1536
R, C = 2, 4  # batch halves x unit quarters
BR = BATCH // R  # 1024 batch rows per core
UC = UNITS // C  # 256 units per core
KS = K // 128  # 12 k-subtiles
M = BR // 128  # 8 batch sub-chunks per core
AW = BR + 1024  # awk row width: at 1024 | wk_lo 512 | wk_hi 512
LO = BR  # wk_lo column offset in awk
HI = BR + 512  # wk_hi column offset in awk

_CACHE = {}


def _build_nc():
    import concourse.tile as tile
    from concourse import bacc, mybir

    f32 = mybir.dt.float32
    f16 = mybir.dt.float16
    Sig = mybir.ActivationFunctionType.Sigmoid
    Tanh = mybir.ActivationFunctionType.Tanh

    nc = bacc.Bacc("TRN2")
    awk_in = nc.declare_dram_parameter("awk", [K, AW], f16, isOutput=False)
    ct_in = nc.declare_dram_parameter("ct", [BR, UC], f32, isOutput=False)
    h_out = nc.declare_dram_parameter("h_out", [BR, UC], f32, isOutput=True)
    c_out = nc.declare_dram_parameter("c_out", [BR, UC], f32, isOutput=True)

    with tile.TileContext(nc) as tc:
        with (
            tc.tile_pool(name="data", bufs=1) as data,
            tc.tile_pool(name="work", bufs=3) as work,
            tc.tile_pool(name="psum", bufs=8, space="PSUM") as psum,
        ):
            awk = data.tile([128, KS, AW], f16)
            ct = data.tile([128, M, UC], f32)
            sig_if = data.tile([128, M, 512], f32)
            fc_all = data.tile([128, M, UC], f32)
            dum_w = data.tile([128, 128], f16)
            dum_a = data.tile([128, 8], f32)
            dum_o = data.tile([128, 8], f32)

            awk_r = awk_in[:].rearrange("(ko p) n -> p ko n", p=128)
            ct_r = ct_in[:].rearrange("(m p) u -> p m u", p=128)

            plo = [
                psum.tile([128, 512], f32, tag="ps", name=f"plo{m}") for m in range(M)
            ]

            # PE p-state warmup: 128-wide dummy matmuls from ~0.5us until
            # real data lands, so real matmuls start at full clock.
            nc.gpsimd.memset(dum_w[:], 0.0)
            for _ in range(32):
                nc.tensor.matmul(
                    plo[0][:, 0:128], dum_w[:], dum_w[:], start=True, stop=True,
                    skip_group_check=True,
                )
            # Act table preload (Sigmoid's table also serves Tanh)
            nc.gpsimd.memset(dum_a[:], 0.0)
            nc.scalar.activation(dum_o[:], dum_a[:], Sig)

            # input stream, Sync queue, consumption order: k0's at+wk_lo
            # first, then full k rows; ct and wk_hi's k0 ride at the end
            # (phase 2 only).
            nc.sync.dma_start(awk[:, 0:1, 0:HI], awk_r[:, 0:1, 0:HI])
            for k in range(1, KS):
                nc.sync.dma_start(awk[:, k : k + 1, :], awk_r[:, k : k + 1, :])
            nc.sync.dma_start(awk[:, 0:1, HI:AW], awk_r[:, 0:1, HI:AW])
            for j in range(2):
                ms4 = slice(4 * j, 4 * j + 4)
                nc.sync.dma_start(ct[:, ms4, :], ct_r[:, ms4, :])

            # phase 1: all m, i|f columns, k-outer round-robin
            for k in range(KS):
                for m in range(M):
                    nc.tensor.matmul(
                        plo[m][:],
                        awk[:, k, m * 128 : (m + 1) * 128],
                        awk[:, k, LO : LO + 512],
                        start=(k == 0),
                        stop=(k == KS - 1),
                    )
            for m in range(M):
                nc.scalar.activation(sig_if[:, m, :], plo[m][:], Sig)
            for m in range(M):
                nc.vector.tensor_mul(
                    fc_all[:, m, :], sig_if[:, m, UC : 2 * UC], ct[:, m, :]
                )

            def g_group(m, pt):
                ms = slice(m * 128, (m + 1) * 128)
                for k in range(KS):
                    nc.tensor.matmul(
                        pt[:, 0:UC],
                        awk[:, k, ms],
                        awk[:, k, HI : HI + UC],
                        start=(k == 0),
                        stop=(k == KS - 1),
                    )

            def o_group(m, pt, osl):
                ms = slice(m * 128, (m + 1) * 128)
                for k in range(KS):
                    nc.tensor.matmul(
                        pt[:, osl],
                        awk[:, k, ms],
                        awk[:, k, HI + UC + osl.start : HI + UC + osl.stop],
                        start=(k == 0),
                        stop=(k == KS - 1),
                    )

            # phase 2a: m7's g-group first; its tanh/ig/cn/c-out/tanh(c)
            # chain completes early, off the kernel tail.
            m7 = M - 1
            ms7 = slice(m7 * 128, (m7 + 1) * 128)
            phg7 = psum.tile([128, 512], f32, tag="ps", name="phg7")
            g_group(m7, phg7)
            tg7 = work.tile([128, UC], f32, tag="tg")
            nc.scalar.activation(tg7[:], phg7[:, 0:UC], Tanh)
            ig7 = work.tile([128, UC], f32, tag="ig")
            nc.vector.tensor_mul(ig7[:], sig_if[:, m7, 0:UC], tg7[:])
            cn7 = work.tile([128, UC], f32, tag="cn")
            nc.vector.tensor_add(cn7[:], fc_all[:, m7, :], ig7[:])
            nc.sync.dma_start(c_out[ms7, :], cn7[:])
            # th7 lives until the kernel tail — keep it out of the rotating
            # work pool (a later th alloc reusing its buffer would deadlock
            # the in-order Act queue against hn7's sigmoid).
            th7 = data.tile([128, UC], f32)
            nc.scalar.activation(th7[:], cn7[:], Tanh)

            # phase 2b: m0..m6, g then o per m, pipelined epilogues
            for m in range(M - 1):
                ms = slice(m * 128, (m + 1) * 128)
                phg = psum.tile([128, 512], f32, tag="ps", name=f"phg{m}")
                g_group(m, phg)
                tg = work.tile([128, UC], f32, tag="tg")
                nc.scalar.activation(tg[:], phg[:, 0:UC], Tanh)
                pho = psum.tile([128, 512], f32, tag="ps", name=f"pho{m}")
                o_group(m, pho, slice(0, UC))
                so = work.tile([128, UC], f32, tag="so")
                nc.scalar.activation(so[:], pho[:, 0:UC], Sig)
                ig = work.tile([128, UC], f32, tag="ig")
                nc.vector.tensor_mul(ig[:], sig_if[:, m, 0:UC], tg[:])
                cn = work.tile([128, UC], f32, tag="cn")
                nc.vector.tensor_add(cn[:], fc_all[:, m, :], ig[:])
                nc.sync.dma_start(c_out[ms, :], cn[:])
                th = work.tile([128, UC], f32, tag="th")
                nc.scalar.activation(th[:], cn[:], Tanh)
                hn = work.tile([128, UC], f32, tag="hn")
                nc.vector.tensor_mul(hn[:], so[:], th[:])
                nc.sync.dma_start(h_out[ms, :], hn[:])

            # phase 2c: m7's o-gate last, two 128-col groups in separate
            # PSUM tiles so each sigmoid fires as its half closes.
            hn7 = work.tile([128, UC], f32, tag="hn")
            for half in range(2):
                osl = slice(128 * half, 128 * (half + 1))
                pho = psum.tile([128, 512], f32, tag="ps", name=f"pho7{half}")
                o_group(m7, pho, osl)
                so = work.tile([128, UC], f32, tag="so")
                nc.scalar.activation(so[:, osl], pho[:, osl], Sig)
                nc.vector.tensor_mul(hn7[:, osl], so[:, osl], th7[:, osl])
                nc.sync.dma_start(h_out[ms7, osl], hn7[:, osl])

    nc.compile()
    return nc


def get_nc():
    if "nc" not in _CACHE:
        _CACHE["nc"] = _build_nc()
    return _CACHE["nc"]


def make_in_maps(inputs, h_tm1, c_tm1, kernel):
    x = np.asarray(inputs, dtype=np.float32)
    h = np.asarray(h_tm1, dtype=np.float32)
    c = np.ascontiguousarray(np.asarray(c_tm1, dtype=np.float32))
    w16 = np.asarray(kernel, dtype=np.float32).astype(np.float16)
    at_full = np.concatenate([h, x], axis=1).T.astype(np.float16)  # [K, B]
    in_maps = []
    for core in range(R * C):
        r, ci = divmod(core, C)
        at_np = at_full[:, r * BR : (r + 1) * BR]
        gates = [
            w16[:, g * UNITS + ci * UC : g * UNITS + (ci + 1) * UC] for g in range(4)
        ]
        awk_np = np.ascontiguousarray(
            np.concatenate([at_np] + gates, axis=1)
        )  # [K, 1024|512|512] fp16
        ct_np = np.ascontiguousarray(c[r * BR : (r + 1) * BR, ci * UC : (ci + 1) * UC])
        in_maps.append({"awk": awk_np, "ct": ct_np})
    return in_maps


def assemble(results):
    h_new = np.empty((BATCH, UNITS), dtype=np.float32)
    c_new = np.empty((BATCH, UNITS), dtype=np.float32)
    for core in range(R * C):
        r, ci = divmod(core, C)
        h_new[r * BR : (r + 1) * BR, ci * UC : (ci + 1) * UC] = results[core]["h_out"]
        c_new[r * BR : (r + 1) * BR, ci * UC : (ci + 1) * UC] = results[core]["c_out"]
    return h_new, c_new


def kernel(inputs, h_tm1, c_tm1, kernel):
    from concourse.bass_utils import run_bass_kernel_spmd

    nc = get_nc()
    in_maps = make_in_maps(inputs, h_tm1, c_tm1, kernel)
    res = run_bass_kernel_spmd(nc, in_maps, list(range(R * C)), trace=False)
    return assemble(res.results)


# revision 9
# speedup vs baseline: 1.2172x; 1.0008x over previous
"""Trainium2 Bass kernel for a custom LSTM cell.

reference:
    z = concat([h_tm1, inputs], -1) @ kernel      # [B, 4U]
    i, f, g, o = split(z, 4, -1)
    c = sigmoid(f) * c_tm1 + sigmoid(i) * tanh(g)
    h = sigmoid(o) * tanh(c)
    returns (h, c)

Sharding over 8 NeuronCores: 2-way over batch x 4-way over units
(each gate's block co-located per core).  Matmul inputs are quantized
to fp16 host-side (h rel-err ~2e-3, well under the 2e-2 budget) which
halves HBM traffic vs fp32r at the same PE rate.

Per core the activations and weights are packed into ONE dram tensor
`awk` [K, 2048] = [at batch 1024 | wk_lo 512 | wk_hi 512] per k-row,
so each k-subtile arrives as a single DMA (descriptor generation at
~650ns/DMA would otherwise outpace the transfers).

Schedule (per core):
  warmup: small dummy matmuls keep the PE busy from ~0.5us so the
          p-state clock is at 2.4GHz when real data lands (~3.9us);
          a dummy Sigmoid preloads the Act table (serves Tanh too).
  phase 1: k-outer round-robin over 8 PSUM groups (batch sub-tiles x
           i|f columns) consuming awk chunks in arrival order; each
           group closes with Sigmoid -> sig_if, then f*c_tm1 on DVE.
  phase 2: m7's g-group runs FIRST so its tanh/ig/cn/tanh(c) chain and
           c_out DMA complete ~17us before the end; then m0..m6
           (g then o per m, separate PSUM tiles to avoid false WAR
           deps), with m7's o-gate last, split into two 128-col groups
           so the final sigmoid->mul->DMA tail is half width.
All input DMAs stream on the Sync queue in consumption order; wk_hi's
k0 block and c_tm1 ride at the end of the stream (first needed by
phase 2).  Outputs also leave on the Sync queue (HWDGE descriptor
generation, keeping Pool's slow SWDGE off the tail).
"""

import sys

sys.path.insert(0, "/opt/trn_rl_repo")

import numpy as np

BATCH, INPUT_DIM, UNITS = 2048, 512, 1024
K = UNITS + INPUT_DIM  # contraction dim, 1536
R, C = 2, 4  # batch halves x unit quarters
BR = BATCH // R  # 1024 batch rows per core
UC = UNITS // C  # 256 units per core
KS = K // 128  # 12 k-subtiles
M = BR // 128  # 8 batch sub-chunks per core
AW = BR + 1024  # awk row width: at 1024 | wk_lo 512 | wk_hi 512
LO = BR  # wk_lo column offset in awk
HI = BR + 512  # wk_hi column offset in awk

_CACHE = {}


def _build_nc():
    import concourse.tile as tile
    from concourse import bacc, mybir

    f32 = mybir.dt.float32
    f16 = mybir.dt.float16
    Sig = mybir.ActivationFunctionType.Sigmoid
    Tanh = mybir.ActivationFunctionType.Tanh

    nc = bacc.Bacc("TRN2")
    awk_in = nc.declare_dram_parameter("awk", [K, AW], f16, isOutput=False)
    ct_in = nc.declare_dram_parameter("ct", [BR, UC], f32, isOutput=False)
    h_out = nc.declare_dram_parameter("h_out", [BR, UC], f32, isOutput=True)
    c_out = nc.declare_dram_parameter("c_out", [BR, UC], f32, isOutput=True)

    with tile.TileContext(nc) as tc:
        with (
            tc.tile_pool(name="data", bufs=1) as data,
            tc.tile_pool(name="work", bufs=3) as work,
            tc.tile_pool(name="psum", bufs=8, space="PSUM") as psum,
        ):
            awk = data.tile([128, KS, AW], f16)
            ct = data.tile([128, M, UC], f32)
            sig_if = data.tile([128, M, 512], f32)
            fc_all = data.tile([128, M, UC], f32)
            dum_w = data.tile([128, 128], f16)
            dum_a = data.tile([128, 8], f32)
            dum_o = data.tile([128, 8], f32)

            awk_r = awk_in[:].rearrange("(ko p) n -> p ko n", p=128)
            ct_r = ct_in[:].rearrange("(m p) u -> p m u", p=128)

            plo = [
                psum.tile([128, 512], f32, tag="ps", name=f"plo{m}") for m in range(M)
            ]

            # PE p-state warmup: 128-wide dummy matmuls from ~0.5us until
            # real data lands, so real matmuls start at full clock.
            nc.gpsimd.memset(dum_w[:], 0.0)
            for _ in range(36):
                nc.tensor.matmul(
                    plo[0][:, 0:128], dum_w[:], dum_w[:], start=True, stop=True,
                    skip_group_check=True,
                )
            # Act table preload (Sigmoid's table also serves Tanh); memset on
            # DVE so it doesn't delay dum_w's memset on the Pool queue
            nc.vector.memset(dum_a[:], 0.0)
            nc.scalar.activation(dum_o[:], dum_a[:], Sig)

            # input stream, Sync queue, consumption order: k0's at+wk_lo
            # first, then full k rows; ct and wk_hi's k0 ride at the end
            # (phase 2 only).
            nc.sync.dma_start(awk[:, 0:1, 0:HI], awk_r[:, 0:1, 0:HI])
            for k in range(1, KS):
                nc.sync.dma_start(awk[:, k : k + 1, :], awk_r[:, k : k + 1, :])
            nc.sync.dma_start(awk[:, 0:1, HI:AW], awk_r[:, 0:1, HI:AW])
            for j in range(2):
                ms4 = slice(4 * j, 4 * j + 4)
                nc.sync.dma_start(ct[:, ms4, :], ct_r[:, ms4, :])

            # phase 1: all m, i|f columns, k-outer round-robin
            for k in range(KS):
                for m in range(M):
                    nc.tensor.matmul(
                        plo[m][:],
                        awk[:, k, m * 128 : (m + 1) * 128],
                        awk[:, k, LO : LO + 512],
                        start=(k == 0),
                        stop=(k == KS - 1),
                    )
            for m in range(M):
                nc.scalar.activation(sig_if[:, m, :], plo[m][:], Sig)
            for m in range(M):
                nc.vector.tensor_mul(
                    fc_all[:, m, :], sig_if[:, m, UC : 2 * UC], ct[:, m, :]
                )

            def g_group(m, pt):
                ms = slice(m * 128, (m + 1) * 128)
                for k in range(KS):
                    nc.tensor.matmul(
                        pt[:, 0:UC],
                        awk[:, k, ms],
                        awk[:, k, HI : HI + UC],
                        start=(k == 0),
                        stop=(k == KS - 1),
                    )

            def o_group(m, pt, osl):
                ms = slice(m * 128, (m + 1) * 128)
                for k in range(KS):
                    nc.tensor.matmul(
                        pt[:, osl],
                        awk[:, k, ms],
                        awk[:, k, HI + UC + osl.start : HI + UC + osl.stop],
                        start=(k == 0),
                        stop=(k == KS - 1),
                    )

            # phase 2a: m7's g-group first; its tanh/ig/cn/c-out/tanh(c)
            # chain completes early, off the kernel tail.
            m7 = M - 1
            ms7 = slice(m7 * 128, (m7 + 1) * 128)
            phg7 = psum.tile([128, 512], f32, tag="ps", name="phg7")
            g_group(m7, phg7)
            tg7 = work.tile([128, UC], f32, tag="tg")
            nc.scalar.activation(tg7[:], phg7[:, 0:UC], Tanh)
            ig7 = work.tile([128, UC], f32, tag="ig")
            nc.vector.tensor_mul(ig7[:], sig_if[:, m7, 0:UC], tg7[:])
            cn7 = work.tile([128, UC], f32, tag="cn")
            nc.vector.tensor_add(cn7[:], fc_all[:, m7, :], ig7[:])
            nc.sync.dma_start(c_out[ms7, :], cn7[:])
            # th7 lives until the kernel tail — keep it out of the rotating
            # work pool (a later th alloc reusing its buffer would deadlock
            # the in-order Act queue against hn7's sigmoid).
            th7 = data.tile([128, UC], f32)
            nc.scalar.activation(th7[:], cn7[:], Tanh)

            # phase 2b: m0..m6, g then o per m, pipelined epilogues
            for m in range(M - 1):
                ms = slice(m * 128, (m + 1) * 128)
                phg = psum.tile([128, 512], f32, tag="ps", name=f"phg{m}")
                g_group(m, phg)
                tg = work.tile([128, UC], f32, tag="tg")
                nc.scalar.activation(tg[:], phg[:, 0:UC], Tanh)
                pho = psum.tile([128, 512], f32, tag="ps", name=f"pho{m}")
                o_group(m, pho, slice(0, UC))
                so = work.tile([128, UC], f32, tag="so")
                nc.scalar.activation(so[:], pho[:, 0:UC], Sig)
                ig = work.tile([128, UC], f32, tag="ig")
                nc.vector.tensor_mul(ig[:], sig_if[:, m, 0:UC], tg[:])
                cn = work.tile([128, UC], f32, tag="cn")
                nc.vector.tensor_add(cn[:], fc_all[:, m, :], ig[:])
                nc.sync.dma_start(c_out[ms, :], cn[:])
                th = work.tile([128, UC], f32, tag="th")
                nc.scalar.activation(th[:], cn[:], Tanh)
                hn = work.tile([128, UC], f32, tag="hn")
                nc.vector.tensor_mul(hn[:], so[:], th[:])
                nc.sync.dma_start(h_out[ms, :], hn[:])

            # phase 2c: m7's o-gate last, two 128-col groups in separate
            # PSUM tiles so each sigmoid fires as its half closes.
            hn7 = work.tile([128, UC], f32, tag="hn")
            for half in range(2):
                osl = slice(128 * half, 128 * (half + 1))
                pho = psum.tile([128, 512], f32, tag="ps", name=f"pho7{half}")
                o_group(m7, pho, osl)
                so = work.tile([128, UC], f32, tag="so")
                nc.scalar.activation(so[:, osl], pho[:, osl], Sig)
                nc.vector.tensor_mul(hn7[:, osl], so[:, osl], th7[:, osl])
            # single h7 DMA: two would serialize 625ns descriptor gens on
            # the shared HWDGE right at the kernel tail
            nc.sync.dma_start(h_out[ms7, :], hn7[:])

    nc.compile()
    return nc


def get_nc():
    if "nc" not in _CACHE:
        _CACHE["nc"] = _build_nc()
    return _CACHE["nc"]


def make_in_maps(inputs, h_tm1, c_tm1, kernel):
    x = np.asarray(inputs, dtype=np.float32)
    h = np.asarray(h_tm1, dtype=np.float32)
    c = np.ascontiguousarray(np.asarray(c_tm1, dtype=np.float32))
    w16 = np.asarray(kernel, dtype=np.float32).astype(np.float16)
    at_full = np.concatenate([h, x], axis=1).T.astype(np.float16)  # [K, B]
    in_maps = []
    for core in range(R * C):
        r, ci = divmod(core, C)
        at_np = at_full[:, r * BR : (r + 1) * BR]
        gates = [
            w16[:, g * UNITS + ci * UC : g * UNITS + (ci + 1) * UC] for g in range(4)
        ]
        awk_np = np.ascontiguousarray(
            np.concatenate([at_np] + gates, axis=1)
        )  # [K, 1024|512|512] fp16
        ct_np = np.ascontiguousarray(c[r * BR : (r + 1) * BR, ci * UC : (ci + 1) * UC])
        in_maps.append({"awk": awk_np, "ct": ct_np})
    return in_maps


def assemble(results):
    h_new = np.empty((BATCH, UNITS), dtype=np.float32)
    c_new = np.empty((BATCH, UNITS), dtype=np.float32)
    for core in range(R * C):
        r, ci = divmod(core, C)
        h_new[r * BR : (r + 1) * BR, ci * UC : (ci + 1) * UC] = results[core]["h_out"]
        c_new[r * BR : (r + 1) * BR, ci * UC : (ci + 1) * UC] = results[core]["c_out"]
    return h_new, c_new


def kernel(inputs, h_tm1, c_tm1, kernel):
    from concourse.bass_utils import run_bass_kernel_spmd

    nc = get_nc()
    in_maps = make_in_maps(inputs, h_tm1, c_tm1, kernel)
    res = run_bass_kernel_spmd(nc, in_maps, list(range(R * C)), trace=False)
    return assemble(res.results)


# revision 10
# speedup vs baseline: 1.2370x; 1.0163x over previous
"""Trainium2 Bass kernel for a custom LSTM cell.

reference:
    z = concat([h_tm1, inputs], -1) @ kernel      # [B, 4U]
    i, f, g, o = split(z, 4, -1)
    c = sigmoid(f) * c_tm1 + sigmoid(i) * tanh(g)
    h = sigmoid(o) * tanh(c)
    returns (h, c)

Sharding over 8 NeuronCores: 2-way over batch x 4-way over units
(each gate's block co-located per core).  Matmul inputs are quantized
to fp16 host-side (h rel-err ~2e-3, well under the 2e-2 budget) which
halves HBM traffic vs fp32r at the same PE rate.

Per core the activations and weights are packed into ONE dram tensor
`awk` [K, 2048] = [at batch 1024 | wk_lo 512 | wk_hi 512] per k-row,
so each k-subtile arrives as a single DMA (descriptor generation at
~650ns/DMA would otherwise outpace the transfers).

Schedule (per core):
  warmup: small dummy matmuls keep the PE busy from ~0.5us so the
          p-state clock is at 2.4GHz when real data lands (~3.9us);
          a dummy Sigmoid preloads the Act table (serves Tanh too).
  phase 1: k-outer round-robin over 8 PSUM groups (batch sub-tiles x
           i|f columns) consuming awk chunks in arrival order; each
           group closes with Sigmoid -> sig_if, then f*c_tm1 on DVE.
  phase 2: m7's g-group runs FIRST so its tanh/ig/cn/tanh(c) chain and
           c_out DMA complete ~17us before the end; then m0..m6
           (g then o per m, separate PSUM tiles to avoid false WAR
           deps), with m7's o-gate last, split into two 128-col groups
           so the final sigmoid->mul->DMA tail is half width.
All input DMAs stream on the Sync queue in consumption order; wk_hi's
k0 block and c_tm1 ride at the end of the stream (first needed by
phase 2).  Outputs also leave on the Sync queue (HWDGE descriptor
generation, keeping Pool's slow SWDGE off the tail).
"""

import sys

sys.path.insert(0, "/opt/trn_rl_repo")

import numpy as np

BATCH, INPUT_DIM, UNITS = 2048, 512, 1024
K = UNITS + INPUT_DIM  # contraction dim, 1536
R, C = 2, 4  # batch halves x unit quarters
BR = BATCH // R  # 1024 batch rows per core
UC = UNITS // C  # 256 units per core
KS = K // 128  # 12 k-subtiles
M = BR // 128  # 8 batch sub-chunks per core
AW = BR + 1024  # awk row width: at 1024 | wk_lo 512 | wk_hi 512
LO = BR  # wk_lo column offset in awk
HI = BR + 512  # wk_hi column offset in awk

_CACHE = {}


def _build_nc():
    import concourse.tile as tile
    from concourse import bacc, mybir

    f32 = mybir.dt.float32
    f16 = mybir.dt.float16
    Sig = mybir.ActivationFunctionType.Sigmoid
    Tanh = mybir.ActivationFunctionType.Tanh

    nc = bacc.Bacc("TRN2")
    awk_in = nc.declare_dram_parameter("awk", [K, AW], f16, isOutput=False)
    ct_in = nc.declare_dram_parameter("ct", [BR, UC], f32, isOutput=False)
    h_out = nc.declare_dram_parameter("h_out", [BR, UC], f32, isOutput=True)
    c_out = nc.declare_dram_parameter("c_out", [BR, UC], f32, isOutput=True)

    with tile.TileContext(nc) as tc:
        with (
            tc.tile_pool(name="data", bufs=1) as data,
            tc.tile_pool(name="work", bufs=3) as work,
            tc.tile_pool(name="psum", bufs=8, space="PSUM") as psum,
        ):
            awk = data.tile([128, KS, AW], f16)
            ct = data.tile([128, M, UC], f32)
            sig_if = data.tile([128, M, 512], f32)
            fc_all = data.tile([128, M, UC], f32)
            dum_w = data.tile([128, 128], f16)
            dum_a = data.tile([128, 8], f32)
            dum_o = data.tile([128, 8], f32)

            awk_r = awk_in[:].rearrange("(ko p) n -> p ko n", p=128)
            ct_r = ct_in[:].rearrange("(m p) u -> p m u", p=128)

            plo = [
                psum.tile([128, 512], f32, tag="ps", name=f"plo{m}") for m in range(M)
            ]

            # PE p-state warmup: 128-wide dummy matmuls from ~0.5us until
            # real data lands, so real matmuls start at full clock.
            nc.gpsimd.memset(dum_w[:], 0.0)
            for _ in range(28):
                nc.tensor.matmul(
                    plo[0][:, 0:128], dum_w[:], dum_w[:], start=True, stop=True,
                    skip_group_check=True,
                )
            # Act table preload (Sigmoid's table also serves Tanh); memset on
            # DVE so it doesn't delay dum_w's memset on the Pool queue
            nc.vector.memset(dum_a[:], 0.0)
            nc.scalar.activation(dum_o[:], dum_a[:], Sig)

            # input stream, Sync queue, consumption order: k0's at+wk_lo
            # first, then full k rows; ct and wk_hi's k0 ride at the end
            # (phase 2 only).
            nc.sync.dma_start(awk[:, 0:1, 0:HI], awk_r[:, 0:1, 0:HI])
            for k in range(1, KS):
                nc.sync.dma_start(awk[:, k : k + 1, :], awk_r[:, k : k + 1, :])
            nc.sync.dma_start(awk[:, 0:1, HI:AW], awk_r[:, 0:1, HI:AW])
            for j in range(2):
                ms4 = slice(4 * j, 4 * j + 4)
                nc.sync.dma_start(ct[:, ms4, :], ct_r[:, ms4, :])

            # phase 1: all m, i|f columns, k-outer round-robin
            for k in range(KS):
                for m in range(M):
                    nc.tensor.matmul(
                        plo[m][:],
                        awk[:, k, m * 128 : (m + 1) * 128],
                        awk[:, k, LO : LO + 512],
                        start=(k == 0),
                        stop=(k == KS - 1),
                    )
            for m in range(M):
                nc.scalar.activation(sig_if[:, m, :], plo[m][:], Sig)
            for m in range(M):
                nc.vector.tensor_mul(
                    fc_all[:, m, :], sig_if[:, m, UC : 2 * UC], ct[:, m, :]
                )

            def g_group(m, pt):
                ms = slice(m * 128, (m + 1) * 128)
                for k in range(KS):
                    nc.tensor.matmul(
                        pt[:, 0:UC],
                        awk[:, k, ms],
                        awk[:, k, HI : HI + UC],
                        start=(k == 0),
                        stop=(k == KS - 1),
                    )

            def o_group(m, pt, osl):
                ms = slice(m * 128, (m + 1) * 128)
                for k in range(KS):
                    nc.tensor.matmul(
                        pt[:, osl],
                        awk[:, k, ms],
                        awk[:, k, HI + UC + osl.start : HI + UC + osl.stop],
                        start=(k == 0),
                        stop=(k == KS - 1),
                    )

            # phase 2a: m7's g-group first; its tanh/ig/cn/c-out/tanh(c)
            # chain completes early, off the kernel tail.
            m7 = M - 1
            ms7 = slice(m7 * 128, (m7 + 1) * 128)
            phg7 = psum.tile([128, 512], f32, tag="ps", name="phg7")
            g_group(m7, phg7)
            tg7 = work.tile([128, UC], f32, tag="tg")
            nc.scalar.activation(tg7[:], phg7[:, 0:UC], Tanh)
            ig7 = work.tile([128, UC], f32, tag="ig")
            nc.vector.tensor_mul(ig7[:], sig_if[:, m7, 0:UC], tg7[:])
            cn7 = work.tile([128, UC], f32, tag="cn")
            nc.vector.tensor_add(cn7[:], fc_all[:, m7, :], ig7[:])
            nc.sync.dma_start(c_out[ms7, :], cn7[:])
            # th7 lives until the kernel tail — keep it out of the rotating
            # work pool (a later th alloc reusing its buffer would deadlock
            # the in-order Act queue against hn7's sigmoid).
            th7 = data.tile([128, UC], f32)
            nc.scalar.activation(th7[:], cn7[:], Tanh)

            # phase 2b: m0..m6, g then o per m, pipelined epilogues
            for m in range(M - 1):
                ms = slice(m * 128, (m + 1) * 128)
                phg = psum.tile([128, 512], f32, tag="ps", name=f"phg{m}")
                g_group(m, phg)
                tg = work.tile([128, UC], f32, tag="tg")
                nc.scalar.activation(tg[:], phg[:, 0:UC], Tanh)
                pho = psum.tile([128, 512], f32, tag="ps", name=f"pho{m}")
                o_group(m, pho, slice(0, UC))
                so = work.tile([128, UC], f32, tag="so")
                nc.scalar.activation(so[:], pho[:, 0:UC], Sig)
                ig = work.tile([128, UC], f32, tag="ig")
                nc.vector.tensor_mul(ig[:], sig_if[:, m, 0:UC], tg[:])
                cn = work.tile([128, UC], f32, tag="cn")
                nc.vector.tensor_add(cn[:], fc_all[:, m, :], ig[:])
                nc.sync.dma_start(c_out[ms, :], cn[:])
                th = work.tile([128, UC], f32, tag="th")
                nc.scalar.activation(th[:], cn[:], Tanh)
                hn = work.tile([128, UC], f32, tag="hn")
                nc.vector.tensor_mul(hn[:], so[:], th[:])
                nc.sync.dma_start(h_out[ms, :], hn[:])

            # phase 2c: m7's o-gate last, two 128-col groups in separate
            # PSUM tiles so each sigmoid fires as its half closes.
            hn7 = work.tile([128, UC], f32, tag="hn")
            for half in range(2):
                osl = slice(128 * half, 128 * (half + 1))
                pho = psum.tile([128, 512], f32, tag="ps", name=f"pho7{half}")
                o_group(m7, pho, osl)
                so = work.tile([128, UC], f32, tag="so")
                nc.scalar.activation(so[:, osl], pho[:, osl], Sig)
                nc.vector.tensor_mul(hn7[:, osl], so[:, osl], th7[:, osl])
            # single h7 DMA: two would serialize 625ns descriptor gens on
            # the shared HWDGE right at the kernel tail
            nc.sync.dma_start(h_out[ms7, :], hn7[:])

    nc.compile()
    return nc


def get_nc():
    if "nc" not in _CACHE:
        _CACHE["nc"] = _build_nc()
    return _CACHE["nc"]


def make_in_maps(inputs, h_tm1, c_tm1, kernel):
    x = np.asarray(inputs, dtype=np.float32)
    h = np.asarray(h_tm1, dtype=np.float32)
    c = np.ascontiguousarray(np.asarray(c_tm1, dtype=np.float32))
    w16 = np.asarray(kernel, dtype=np.float32).astype(np.float16)
    at_full = np.concatenate([h, x], axis=1).T.astype(np.float16)  # [K, B]
    in_maps = []
    for core in range(R * C):
        r, ci = divmod(core, C)
        at_np = at_full[:, r * BR : (r + 1) * BR]
        gates = [
            w16[:, g * UNITS + ci * UC : g * UNITS + (ci + 1) * UC] for g in range(4)
        ]
        awk_np = np.ascontiguousarray(
            np.concatenate([at_np] + gates, axis=1)
        )  # [K, 1024|512|512] fp16
        ct_np = np.ascontiguousarray(c[r * BR : (r + 1) * BR, ci * UC : (ci + 1) * UC])
        in_maps.append({"awk": awk_np, "ct": ct_np})
    return in_maps


def assemble(results):
    h_new = np.empty((BATCH, UNITS), dtype=np.float32)
    c_new = np.empty((BATCH, UNITS), dtype=np.float32)
    for core in range(R * C):
        r, ci = divmod(core, C)
        h_new[r * BR : (r + 1) * BR, ci * UC : (ci + 1) * UC] = results[core]["h_out"]
        c_new[r * BR : (r + 1) * BR, ci * UC : (ci + 1) * UC] = results[core]["c_out"]
    return h_new, c_new


def kernel(inputs, h_tm1, c_tm1, kernel):
    from concourse.bass_utils import run_bass_kernel_spmd

    nc = get_nc()
    in_maps = make_in_maps(inputs, h_tm1, c_tm1, kernel)
    res = run_bass_kernel_spmd(nc, in_maps, list(range(R * C)), trace=False)
    return assemble(res.results)
